# revision 1
# baseline (speedup 1.0000x reference)
"""Trainium2 Bass kernel for DomainClassMixAugmentation.

Math: the four channel masks (cs&ds, cs&di, cg&ds, cg&di) partition the
(b, c) plane, so the whole module collapses to

    out[b] = A[b,c] * x[b] + Bs[b,c] * x[same_idx[b]] + Bd[b,c] * x[diff_idx[b]]

with per-(sample, channel) scalar coefficients

    A  = s0 where cs&ds, s1 where cg&ds, 1 elsewhere
    Bs = (1-s0) * (cs&ds)[same_idx]
    Bd = (1-s1) * (cg&ds)[diff_idx]

Sharding: spatially over H (56 rows -> 7 rows per core, 8 cores); every
core holds all 32 samples for its spatial slice, so the cross-sample
gathers are purely host-side index remapping of the per-core slices.

Two device launches:
  A) stream x, class_gradient, domain_gradient slices; fused
     multiply+reduce (scalar_tensor_tensor with accum_out) -> per-core
     partial sums of cim/dim over the spatial slice.
  B) one f32 matmul per 4-channel group: a host-built [128,128]
     lane-interleaved matrix W folds the diagonal A term and both
     one-hot gathers, so PE does gather+scale+sum in one pass
     (PSUM -> ScalarE copy -> DMA out).
Host in between: sum the 8 partial [32,256] blocks, take the two
per-sample quantiles (exactly mirroring jnp.quantile's f32 linear
interpolation), form masks, coefficients, and W.
"""

import hashlib
import os
import time

import numpy as np

import concourse.bacc as bacc
import concourse.bass as bass
import concourse.mybir as mybir
import concourse.tile as tile
from concourse import bass2jax

_NEFF_CACHE_DIR = os.path.join(
    os.path.expanduser("~"), ".cache", "bass_neff_cache"
)


def _install_cached_hook():
    """bass2jax's neuronx_cc hook recompiles the NEFF (minutes) on every
    fresh process; wrap it with a content-addressed disk cache."""
    bass2jax.install_neuronx_cc_hook()
    try:
        import libneuronxla
    except ImportError:
        return
    if getattr(libneuronxla, "_ant_disk_cache", False):
        return
    orig = libneuronxla.neuronx_cc
    os.makedirs(_NEFF_CACHE_DIR, exist_ok=True)

    def canonical(code):
        # the raw HLO embeds per-op source_file/source_line metadata, so the
        # same kernel run from a different path/line offset would re-key;
        # strip it before hashing.
        try:
            import libneuronxla.proto.hlo_pb2 as hlo_pb2

            p = hlo_pb2.HloModuleProto.FromString(bytes(code))
            for field in ("stack_frame_index",):
                try:
                    p.ClearField(field)
                except ValueError:
                    pass
            for comp in p.computations:
                for ins in comp.instructions:
                    ins.ClearField("metadata")
            return p.SerializeToString(deterministic=True)
        except Exception:
            return bytes(code)

    def cached(code, code_format, platform_version, file_prefix):
        key = hashlib.sha256(
            b"|".join(
                [canonical(code), bytes(code_format), str(platform_version).encode()]
            )
        ).hexdigest()
        path = os.path.join(_NEFF_CACHE_DIR, key + ".bin")
        if os.path.exists(path):
            with open(path, "rb") as f:
                return 0, f.read()
        ret, data = orig(code, code_format, platform_version, file_prefix)
        if ret == 0 and isinstance(data, bytes) and len(data) > 0:
            tmp = path + f".tmp{os.getpid()}"
            with open(tmp, "wb") as f:
                f.write(data)
            os.replace(tmp, path)
        return ret, data

    libneuronxla.neuronx_cc = cached
    libneuronxla._ant_disk_cache = True

B, C, H, W = 32, 256, 56, 56
NCORES = 8
SH = H // NCORES          # 7 rows of H per core
SP = SH * W               # 392 spatial elements per core per (b, c)
HALVES = C // 128         # 2 partition blocks of channels
NT = B * HALVES           # 64 tiles of [128, SP] per tensor per core
F32 = mybir.dt.float32
AOP = mybir.AluOpType

_CACHE: dict = {}


def _build_reduce_nc():
    """Launch A: per-core partial sums of x*cg and x*dg over the spatial slice.

    Outputs cimp/dimp [128, NT] with column j = b*HALVES + h holding the
    per-channel partial sums for sample b, channel block h.
    """
    nc = bacc.Bacc("TRN2", target_bir_lowering=False, debug=False)
    x = nc.dram_tensor("x", [B, C, SP], F32, kind="ExternalInput").ap()
    cg = nc.dram_tensor("cg", [B, C, SP], F32, kind="ExternalInput").ap()
    dg = nc.dram_tensor("dg", [B, C, SP], F32, kind="ExternalInput").ap()
    cimp = nc.dram_tensor("cimp", [128, NT], F32, kind="ExternalOutput").ap()
    dimp = nc.dram_tensor("dimp", [128, NT], F32, kind="ExternalOutput").ap()

    NB = 4  # samples per DMA batch (fewer, bigger DMAs)
    with tile.TileContext(nc) as tc:
        with (
            tc.tile_pool(name="io", bufs=2) as io,
            tc.tile_pool(name="scr", bufs=3) as scr,
            tc.tile_pool(name="acc", bufs=1) as acc,
        ):
            cims = acc.tile([128, NT], F32, tag="cims")
            dims = acc.tile([128, NT], F32, tag="dims")
            for bb in range(0, B, NB):
                tiles = {}
                for name, src in (("xt", x), ("ct", cg), ("gt", dg)):
                    t = io.tile([128, NB * HALVES * SP], F32, tag=name)
                    nc.sync.dma_start(
                        t[:].rearrange("p (b h n) -> p b h n", b=NB, h=HALVES),
                        src[bb:bb + NB].rearrange("b (h p) n -> p b h n", p=128),
                    )
                    tiles[name] = t
                for lb in range(NB):
                    for h in range(HALVES):
                        j = (bb + lb) * HALVES + h
                        fsl = slice((lb * HALVES + h) * SP, (lb * HALVES + h + 1) * SP)
                        # fused multiply + free-dim reduce: out = (x bypass 0)*g,
                        # accum = sum(out).  (tensor_tensor_reduce traps on HW.)
                        s1 = scr.tile([128, SP], F32, tag="s1")
                        nc.vector.scalar_tensor_tensor(
                            out=s1[:], in0=tiles["xt"][:, fsl], scalar=0.0,
                            in1=tiles["ct"][:, fsl],
                            op0=AOP.bypass, op1=AOP.mult,
                            accum_out=cims[:, j:j + 1],
                        )
                        s2 = scr.tile([128, SP], F32, tag="s2")
                        nc.vector.scalar_tensor_tensor(
                            out=s2[:], in0=tiles["xt"][:, fsl], scalar=0.0,
                            in1=tiles["gt"][:, fsl],
                            op0=AOP.bypass, op1=AOP.mult,
                            accum_out=dims[:, j:j + 1],
                        )
            nc.sync.dma_start(cimp, cims[:])
            nc.sync.dma_start(dimp, dims[:])
    nc.compile()
    return nc


NTC = C // 4  # 64 channel-groups of 4; one block-diag matmul each


def _build_apply_nc():
    """Launch B: out[b, c, :] = sum_j W[c, j, b] * x[j, c, :] via PE.

    W (host-built) folds the diagonal A term and the same/diff one-hot
    gather terms into one [B, B] matrix per channel.  Channels are packed
    4 per matmul: lhsT/rhs partition k = j*4 + cc, out partition
    m = b*4 + cc, with W nonzero only where the cc lanes match.  Plain
    f32 matmul (4 cyc/row) stays under the DMA roofline; products against
    the exact-zero off-lanes are exact.
    """
    nc = bacc.Bacc("TRN2", target_bir_lowering=False, debug=False)
    x = nc.dram_tensor("x", [B, C, SP], F32, kind="ExternalInput").ap()
    w = nc.dram_tensor("w", [128, NTC * 128], F32, kind="ExternalInput").ap()
    out = nc.dram_tensor("out", [B, C, SP], F32, kind="ExternalOutput").ap()

    with tile.TileContext(nc) as tc:
        with (
            tc.tile_pool(name="wp", bufs=1) as wp,
            tc.tile_pool(name="io", bufs=4) as io,
            tc.tile_pool(name="ps", bufs=4, space="PSUM") as ps,
            tc.tile_pool(name="ob", bufs=4) as ob,
        ):
            ws = wp.tile([128, NTC * 128], F32, tag="w")
            nc.sync.dma_start(ws[:], w)

            for q in range(NTC):
                # [B, 4, SP] DRAM slice enumerates (j, c, n) — matches the
                # [128, SP] tile's partition index k = j*4 + c.
                rt = io.tile([128, SP], F32, tag="rt")
                nc.sync.dma_start(rt[:], x[:, q * 4:(q + 1) * 4, :])
                pt = ps.tile([128, SP], F32, tag="pt")
                nc.tensor.matmul(
                    pt[:],
                    lhsT=ws[:, q * 128:(q + 1) * 128],
                    rhs=rt[:],
                    start=True, stop=True,
                )
                ot = ob.tile([128, SP], F32, tag="ot")
                nc.scalar.copy(ot[:], pt[:])
                nc.sync.dma_start(out[:, q * 4:(q + 1) * 4, :], ot[:])
    nc.compile()
    return nc


def _get_nc(key):
    if key not in _CACHE:
        _CACHE[key] = _build_reduce_nc() if key == "reduce" else _build_apply_nc()
    return _CACHE[key]


class _Runner:
    """Cached PJRT runner for a compiled Bass module (8-core SPMD).

    Mirrors bass2jax.run_bass_via_pjrt's multi-core path, but keeps the
    jitted executable (so repeat calls don't re-trace), accepts
    pre-uploaded device arrays, and materialises the donated output
    buffers on device instead of uploading host zeros.
    """

    def __init__(self, nc, n_cores=NCORES):
        import jax
        import jax.numpy as jnp
        from jax.experimental.shard_map import shard_map
        from jax.sharding import Mesh, NamedSharding, PartitionSpec

        _install_cached_hook()
        self.n_cores = n_cores
        pid_name = nc.partition_id_tensor.name if nc.partition_id_tensor else None
        in_names, out_names, out_avals = [], [], []
        for alloc in nc.m.functions[0].allocations:
            if not isinstance(alloc, mybir.MemoryLocationSet):
                continue
            name = alloc.memorylocations[0].name
            if alloc.kind == "ExternalInput":
                if name != pid_name:
                    in_names.append(name)
            elif alloc.kind == "ExternalOutput":
                out_names.append(name)
                out_avals.append(
                    jax.core.ShapedArray(
                        tuple(alloc.tensor_shape), mybir.dt.np(alloc.dtype)
                    )
                )
        self.in_names = in_names
        self.out_names = out_names
        self.out_avals = out_avals
        n_params = len(in_names)
        bind_names = list(in_names) + list(out_names)
        if pid_name is not None:
            bind_names.append(pid_name)

        def _body(*args):
            operands = list(args)
            if pid_name is not None:
                operands.append(bass2jax.partition_id_tensor())
            return tuple(
                bass2jax._bass_exec_p.bind(
                    *operands,
                    out_avals=tuple(out_avals),
                    in_names=tuple(bind_names),
                    out_names=tuple(out_names),
                    lowering_input_output_aliases=(),
                    sim_require_finite=True,
                    sim_require_nnan=True,
                    nc=nc,
                )
            )

        mesh = Mesh(np.asarray(jax.devices()[:n_cores]), ("core",))
        self.sharding = NamedSharding(mesh, PartitionSpec("core"))
        n_outs = len(out_names)
        self._sharded = jax.jit(
            shard_map(
                _body,
                mesh=mesh,
                in_specs=(PartitionSpec("core"),) * (n_params + n_outs),
                out_specs=(PartitionSpec("core"),) * n_outs,
                check_rep=False,
            ),
            donate_argnums=tuple(range(n_params, n_params + n_outs)),
            keep_unused=True,
        )
        self._zeros = jax.jit(
            lambda: tuple(
                jnp.zeros((n_cores * a.shape[0], *a.shape[1:]), a.dtype)
                for a in out_avals
            ),
            out_shardings=tuple(self.sharding for _ in out_avals),
        )

    def put(self, per_core_arrays):
        """Upload a list of per-core np arrays as one sharded device array."""
        import jax

        return jax.device_put(np.concatenate(per_core_arrays, axis=0), self.sharding)

    def put_replicated(self, arr):
        import jax

        return jax.device_put(
            np.concatenate([arr] * self.n_cores, axis=0), self.sharding
        )

    def __call__(self, *device_args):
        """Run with device (or host) args in in_names order; returns jax arrays."""
        return self._sharded(*device_args, *self._zeros())


def _get_runner(key):
    rkey = key + "_runner"
    if rkey not in _CACHE:
        _CACHE[rkey] = _Runner(_get_nc(key))
    return _CACHE[rkey]


def _quantile_f32(sorted_vals, q):
    """jnp.quantile (method='linear') on pre-sorted f32 rows, f32 arithmetic."""
    n = sorted_vals.shape[1]
    qf = np.float32(q) * np.float32(n - 1)
    low = int(np.floor(qf))
    high = int(np.ceil(qf))
    hw = np.float32(qf - np.float32(low))
    lw = np.float32(np.float32(1.0) - hw)
    return sorted_vals[:, low] * lw + sorted_vals[:, high] * hw


def kernel(**inputs):
    x = np.ascontiguousarray(np.asarray(inputs["x"], dtype=np.float32))
    cg = np.ascontiguousarray(np.asarray(inputs["class_gradient"], dtype=np.float32))
    dg = np.ascontiguousarray(np.asarray(inputs["domain_gradient"], dtype=np.float32))
    ms = np.asarray(inputs["mixup_strength"], dtype=np.float32)
    same_idx = np.asarray(inputs["same_idx"]).astype(np.int64)
    diff_idx = np.asarray(inputs["diff_idx"]).astype(np.int64)

    # ---- spatial shards: core k gets rows [k*SH, (k+1)*SH) of H ----------
    def shards(t):
        # [B, C, H, W] -> per-core [B, C, SP] contiguous
        return [
            np.ascontiguousarray(t[:, :, k * SH:(k + 1) * SH, :]).reshape(B, C, SP)
            for k in range(NCORES)
        ]

    x_sl = shards(x)
    cg_sl = shards(cg)
    dg_sl = shards(dg)

    times = {}

    # ---- launch A: partial importance sums -------------------------------
    ra = _get_runner("reduce")
    t0 = time.perf_counter()
    x_dev = ra.put(x_sl)
    cg_dev = ra.put(cg_sl)
    dg_dev = ra.put(dg_sl)
    times["upload_a"] = time.perf_counter() - t0
    t0 = time.perf_counter()
    for attempt in range(3):
        try:
            outs_a = ra(x_dev, cg_dev, dg_dev)
            partials = {
                name: np.asarray(arr).reshape(NCORES, 128, NT)
                for name, arr in zip(ra.out_names, outs_a)
            }
            break
        except Exception:
            # transient NRT/axon exec failures happen; re-upload and retry
            if attempt == 2:
                raise
            time.sleep(2.0)
            x_dev = ra.put(x_sl)
            cg_dev = ra.put(cg_sl)
            dg_dev = ra.put(dg_sl)
    times["exec_a"] = time.perf_counter() - t0

    # partials [NCORES, 128, NT] -> [B, C]; sum the 8 cores in f32
    def unpack(name):
        acc = np.zeros((128, NT), dtype=np.float32)
        for k in range(NCORES):
            acc = acc + partials[name][k]
        # column j = b*HALVES + h ; row p = channel within block
        # cim[b, h*128 + p] = acc[p, b*HALVES + h]
        return (
            acc.reshape(128, B, HALVES)
            .transpose(1, 2, 0)
            .reshape(B, C)
            .astype(np.float32)
        )

    inv_n = np.float32(1.0) / np.float32(H * W)
    cim = unpack("cimp") * inv_n
    dim = unpack("dimp") * inv_n

    # ---- host: quantiles, masks, coefficients ----------------------------
    cim_sorted = np.sort(cim, axis=1)
    dim_sorted = np.sort(dim, axis=1)
    cthr = _quantile_f32(cim_sorted, 0.5)[:, None]
    dthr = _quantile_f32(dim_sorted, 0.8)[:, None]
    cs = cim > cthr
    ds = dim > dthr
    m1 = cs & ds          # class-salient & domain-salient
    m3 = (~cs) & ds       # class-generic & domain-salient

    s0 = ms[:, 0].astype(np.float32)[:, None]
    s1 = ms[:, 1].astype(np.float32)[:, None]
    one = np.float32(1.0)

    A = np.where(m1, s0, np.where(m3, s1, one)).astype(np.float32)
    Bs = np.where(m1[same_idx], one - s0, np.float32(0.0)).astype(np.float32)
    Bd = np.where(m3[diff_idx], one - s1, np.float32(0.0)).astype(np.float32)

    # per-channel mixing matrix Wc[c, j, b]: out[b,c] = sum_j Wc[c,j,b]*x[j,c]
    Wc = np.zeros((C, B, B), dtype=np.float32)
    bi = np.arange(B)
    np.add.at(Wc, (slice(None), bi, bi), A.T)
    np.add.at(Wc, (slice(None), same_idx, bi), Bs.T)
    np.add.at(Wc, (slice(None), diff_idx, bi), Bd.T)
    # pack 4 channels per [128, 128] lhsT, interleaved-diagonal:
    # k = j*4+cc, m = b*4+cc  (channel cc of group q lives on stride-4 lanes)
    Wr = Wc.reshape(NTC, 4, B, B)
    Wblk = np.zeros((NTC, 128, 128), dtype=np.float32)
    for cc in range(4):
        Wblk[:, cc::4, cc::4] = Wr[:, cc]
    # device layout [k, q*128+m]
    Wt = np.ascontiguousarray(Wblk.transpose(1, 0, 2).reshape(128, NTC * 128))

    # ---- launch B: gather + mix via per-channel-group matmuls ------------
    rb = _get_runner("apply")
    t0 = time.perf_counter()
    w_dev = rb.put_replicated(Wt)
    times["upload_b"] = time.perf_counter() - t0
    t0 = time.perf_counter()
    for attempt in range(3):
        try:
            outs_b = rb(x_dev, w_dev)
            out_all = np.asarray(outs_b[0]).reshape(NCORES, B, C, SH, W)
            break
        except Exception:
            if attempt == 2:
                raise
            time.sleep(2.0)
            x_dev = ra.put(x_sl)
            w_dev = rb.put_replicated(Wt)
    times["exec_b"] = time.perf_counter() - t0

    out = np.empty((B, C, H, W), dtype=np.float32)
    for k in range(NCORES):
        out[:, :, k * SH:(k + 1) * SH, :] = out_all[k]
    _CACHE["last_times"] = times
    return out

